# revision 1
# baseline (speedup 1.0000x reference)
"""Distributed TRN2 kernel for nn_CustomFullyConnectedLayerSoftmax.

Math: the reference's scatter-add builds W[r, c] = V_scaled[(r-c) % 2048, c]
(each (r, c) hit exactly once -> pure permutation), then out = x @ W.T.
So out[:, r] needs column r of W.T, i.e. W.T[c, r] = V_scaled[(r-c)%2048, c].

Sharding: output columns r are split across 8 cores (256 each). Core i
receives B_i = W.T[:, 256*i : 256*(i+1)] as a dense [2048, 256] operand,
pre-arranged on host into the SBUF layout [128 partitions, 16 k-chunks, 256],
plus the replicated x.T in layout [128, 16, 32]. Each core computes its
disjoint out[:, 256*i:256*(i+1)] = x @ B_i with 16 accumulating matmuls --
no collectives; host concatenates the 8 slices.

Device traffic per core: B_i + xT = its 1/8 share of V plus a replicated x,
which is the memory roofline for this op.
"""

import numpy as np

from concourse import bacc, mybir, tile
from concourse import bass_utils

IN_F = 2048
OUT_F = 2048
TOTAL = 2048
BATCH = 32
N_CORES = 8
R_SH = OUT_F // N_CORES          # 256 output columns per core
K_CH = IN_F // 128               # 16 contraction chunks of 128
K_TOPK = 1844                    # ceil(int(0.9 * 2048 * 2048) / 2048)

# 'f32' or 'bf16' compute/storage dtype for the matmul operands.
DEVICE_DTYPE = "f32"
# Number of B sub-DMAs (each covers K_CH // N_SPLITS k-chunks) for
# DMA/matmul overlap.
N_SPLITS = 4

TRACE = False          # set True (from test.py) to capture neuron-profile
TRACE_KWARGS = {}
LAST_RESULT = None     # BassKernelResults of the most recent run

_graph_cache = {}


def _mybir_dt(key):
    return mybir.dt.float32 if key == "f32" else mybir.dt.bfloat16


def _np_dt(key):
    return mybir.dt.np(_mybir_dt(key))


def _build_graph(dtype_key):
    dt = _mybir_dt(dtype_key)
    nc = bacc.Bacc("TRN2", target_bir_lowering=False, debug=False)

    xT_d = nc.dram_tensor("xT", [128, K_CH, BATCH], dt, kind="ExternalInput")
    B_d = nc.dram_tensor("B", [128, K_CH, R_SH], dt, kind="ExternalInput")
    out_d = nc.dram_tensor("out", [BATCH, R_SH], mybir.dt.float32,
                           kind="ExternalOutput")

    kper = K_CH // N_SPLITS
    with tile.TileContext(nc) as tc:
        with (
            tc.tile_pool(name="xpool", bufs=1) as xpool,
            tc.tile_pool(name="bpool", bufs=N_SPLITS) as bpool,
            tc.tile_pool(name="opool", bufs=1) as opool,
            tc.tile_pool(name="psum", bufs=1, space="PSUM") as pspool,
        ):
            xt = xpool.tile([128, K_CH, BATCH], dt)
            nc.sync.dma_start(xt[:], xT_d[:])
            acc = pspool.tile([BATCH, R_SH], mybir.dt.float32)
            for j in range(N_SPLITS):
                bt = bpool.tile([128, kper, R_SH], dt, tag="bt")
                nc.sync.dma_start(bt[:], B_d[:, j * kper:(j + 1) * kper, :])
                for k in range(kper):
                    kk = j * kper + k
                    nc.tensor.matmul(
                        acc[:],
                        xt[:, kk, :],
                        bt[:, k, :],
                        start=(kk == 0),
                        stop=(kk == K_CH - 1),
                    )
            ot = opool.tile([BATCH, R_SH], mybir.dt.float32)
            nc.vector.tensor_copy(ot[:], acc[:])
            nc.sync.dma_start(out_d[:], ot[:])

    nc.compile()
    return nc


def _get_graph(dtype_key):
    if dtype_key not in _graph_cache:
        _graph_cache[dtype_key] = _build_graph(dtype_key)
    return _graph_cache[dtype_key]


def _host_shards(x, V, alpha, dtype_key):
    np_dt = _np_dt(dtype_key)

    a = alpha.astype(np.float64)
    e = np.exp(a - a.max())
    scale = np.clip(K_TOPK * (e / e.sum()), 0.0, 1.0).astype(np.float32)
    Vs = V * scale[:, None]                        # [2048, 2048] f32

    # W.T[c, r] = Vs[(r - c) % 2048, c]; with Vt = Vs.T duplicated along
    # columns, row c of W.T is the window Vt2[c, 2048-c : 4096-c] -> a
    # shear expressible as a strided view of the flat buffer.
    Vt2 = np.concatenate([Vs.T, Vs.T], axis=1)     # [2048, 4096]
    flat = np.ascontiguousarray(Vt2).reshape(-1)
    WT = np.lib.stride_tricks.as_strided(
        flat[TOTAL:], shape=(IN_F, OUT_F),
        strides=((2 * TOTAL - 1) * 4, 4))

    xT = np.ascontiguousarray(x.T)                 # [2048, 32]
    xT_dev = np.ascontiguousarray(
        xT.reshape(K_CH, 128, BATCH).transpose(1, 0, 2)).astype(np_dt)

    in_maps = []
    for i in range(N_CORES):
        Bi = np.asarray(WT[:, i * R_SH:(i + 1) * R_SH])   # [2048, 256]
        Bi_dev = np.ascontiguousarray(
            Bi.reshape(K_CH, 128, R_SH).transpose(1, 0, 2)).astype(np_dt)
        in_maps.append({"xT": xT_dev, "B": Bi_dev})
    return in_maps


def kernel(x, V, alpha):
    global LAST_RESULT
    x = np.asarray(x, dtype=np.float32)
    V = np.asarray(V, dtype=np.float32)
    alpha = np.asarray(alpha, dtype=np.float32)

    in_maps = _host_shards(x, V, alpha, DEVICE_DTYPE)
    nc = _get_graph(DEVICE_DTYPE)
    res = bass_utils.run_bass_kernel_spmd(
        nc, in_maps, core_ids=list(range(N_CORES)),
        trace=TRACE, trace_kwargs=TRACE_KWARGS)
    LAST_RESULT = res
    out = np.concatenate([np.asarray(r["out"]) for r in res.results], axis=1)
    return np.ascontiguousarray(out, dtype=np.float32)
